# revision 25
# baseline (speedup 1.0000x reference)
"""GAT layer (multi-head graph attention) on 8 TRN2 NeuronCores.

Strategy: destination nodes are sharded across the 8 cores.  Each core:
  phase 1: computes the full projection table redundantly (bf16 GEMM
           X @ W.T plus the per-head attention score reductions), packed
           as [proj bf16 (1024B) | s_src f32 (32B) | s_tgt f32 (32B)]
           rows in local HBM (1280B gather stride, 1088B written).
  phase 1.5: loads the core's own shard's s_tgt resident in SBUF as
           hi/lo bf16 pairs (f32-accurate sums via two bf16 adds).
  phase 2: walks 49 pairs of 128-target windows.  Edges are pre-sorted
           by (window-pair, src-bucket) on the host; merged dma_gather
           calls (<=1024 int16 indices per 32768-row bucket) pull
           source rows; the slot->dst scatter one-hot is built on-device
           (DVE is_equal of a streamed dst-id against an iota ramp), the
           transposed one-hot for s_tgt expansion streams from host as
           fp8; PE matmuls route scatter-adds; softmax division + PReLU
           at window flush.  (-1 index skipping and a second 256B-row
           gather both crash the gather ucode on this stack; see git
           history of this file for the guarded attempts.)

kernel(**inputs) takes the FULL inputs and returns the FULL output.
"""

import math
from dataclasses import dataclass, field

import numpy as np
import ml_dtypes

BF16 = ml_dtypes.bfloat16
P = 128
GMAX = 1024  # dma_gather num_idxs hard cap (Q7 scratch limit)


def _ceil(a, b):
    return -(-a // b)


@dataclass
class Cfg:
    N: int = 100000
    E: int = 800000
    HID: int = 512
    HEADS: int = 8
    ncores: int = 8
    bucket: int = 32768
    leak: float = 0.01
    merge: bool = True    # merge window pairs into one gather call per bucket
    negpad: bool = False  # -1 trailing pads crash the gather ucode; keep 0

    def __post_init__(self):
        assert self.N % self.ncores == 0
        self.F = self.HID // self.HEADS
        self.shard = self.N // self.ncores
        self.NW = _ceil(self.shard, P)          # windows per core
        assert self.NW % 2 == 0
        self.NP2 = self.NW // 2                 # window pairs
        self.NB = _ceil(self.N, self.bucket)    # src buckets (int16 range)
        self.NT = _ceil(self.N, P)              # projection tiles
        self.NPAD = self.NT * P
        self.KP = min(self.HID, P)              # contraction partitions
        self.KT = self.HID // self.KP           # contraction tiles
        self.row_bf = 640                       # gather stride in bf16 elems (1280B)
        self.row_used = 544                     # written cols: 512 proj + 16 f32 scores
        self.s_src_f32 = 256                    # f32 col of s_src within a row
        self.s_tgt_f32 = 264


@dataclass
class Schedule:
    """Core-independent (uniform) phase-2 schedule over window pairs."""
    seg: np.ndarray       # [NW, NB] slot counts (128-aligned, global max)
    TW: list              # tiles per window
    T2: list              # tiles per pair
    T2max: int
    TT: int               # total tiles
    g1calls: list         # per pair: [(b, tile_off, [(w, nslots)...], idxcol0)]
    g2calls: list         # per pair: [(tile_off, ntiles, idxcol2)]
    idxcols1: int
    idxcols2: int
    tile_base: list       # first global tile of each pair
    wtiles: list          # per window: local tile offsets within its pair
    tinfo: list           # per global tile: (w, b, slot0) slot0 = first slot of
                          # this tile within its (w, b) segment


def build_schedule(cfg: Cfg, counts: np.ndarray) -> Schedule:
    """counts: [ncores, NW, NB] edge counts."""
    maxcnt = counts.max(axis=0)  # [NW, NB]
    seg = np.where(maxcnt > 0, _ceil(maxcnt, P) * P, 0).astype(np.int64)
    TW = [int(seg[w].sum()) // P for w in range(cfg.NW)]
    g1calls, g2calls, tile_base = [], [], []
    wtiles = [[] for _ in range(cfg.NW)]
    tinfo = []
    T2l = []
    icol1 = icol2 = 0
    tt = 0
    for pr in range(cfg.NP2):
        w0, w1 = 2 * pr, 2 * pr + 1
        tile_base.append(tt)
        pcalls = []
        toff = 0
        for b in range(cfg.NB):
            n0, n1 = int(seg[w0, b]), int(seg[w1, b])
            if n0 + n1 == 0:
                continue
            if cfg.merge and n0 + n1 <= GMAX and n0 > 0 and n1 > 0:
                groups = [[(w0, n0), (w1, n1)]]
            else:
                groups = [[(w, n)] for (w, n) in ((w0, n0), (w1, n1)) if n > 0]
            for g in groups:
                n = sum(x[1] for x in g)
                assert n <= GMAX, f"gather call too large: {n}"
                pcalls.append((b, toff, g, icol1))
                for (w, nw) in g:
                    for j in range(nw // P):
                        wtiles[w].append(toff)
                        tinfo.append((w, b, j * P))
                        toff += 1
                icol1 += n // 16
        g1calls.append(pcalls)
        gcalls = []
        for c0 in range(0, toff, GMAX // P):
            nt = min(GMAX // P, toff - c0)
            gcalls.append((c0, nt, icol2))
            icol2 += nt * 8
        g2calls.append(gcalls)
        T2l.append(toff)
        tt += toff
    return Schedule(seg=seg, TW=TW, T2=T2l, T2max=max(T2l), TT=tt,
                    g1calls=g1calls, g2calls=g2calls,
                    idxcols1=icol1, idxcols2=icol2,
                    tile_base=tile_base, wtiles=wtiles, tinfo=tinfo)


def prep_core(cfg: Cfg, sched: Schedule, src, trg, k):
    """Per-core input arrays: g1i/g2i idx streams and dstp (dst-local ids)."""
    mask = (trg // cfg.shard) == k
    esrc = src[mask]
    etrg = trg[mask]
    trel = etrg - k * cfg.shard
    win = trel // P
    buck = esrc // cfg.bucket
    order = np.lexsort((buck, win))
    esrc, trel, win, buck = (a[order] for a in (esrc, trel, win, buck))
    dstid = trel % P

    key = win * cfg.NB + buck
    starts = np.searchsorted(key, np.arange(cfg.NW * cfg.NB), side="left")
    ends = np.searchsorted(key, np.arange(cfg.NW * cfg.NB), side="right")

    g1i = np.zeros((P, sched.idxcols1), np.int16)
    ohd1 = np.zeros((P, sched.TT, P), ml_dtypes.float8_e4m3)  # [dst, tile, slot]
    dstp = np.full((P, sched.TT), -1.0, BF16)

    def wrap16(idx):
        blk = idx.reshape(len(idx) // 16, 16).T  # [16, cols]
        return np.tile(blk, (8, 1))

    for pr in range(cfg.NP2):
        tb = sched.tile_base[pr]
        for (b, toff, groups, icol1) in sched.g1calls[pr]:
            n = sum(x[1] for x in groups)
            idx = np.zeros(n, np.int16)
            off = 0
            for gi, (w, nw) in enumerate(groups):
                lo, hi = int(starts[w * cfg.NB + b]), int(ends[w * cfg.NB + b])
                cnt = hi - lo
                assert cnt <= nw
                idx[off:off + cnt] = (esrc[lo:hi] - b * cfg.bucket).astype(np.int16)
                if cfg.negpad and gi == len(groups) - 1:
                    # 0-pad to a 16-aligned boundary, then -1 (ucode skips the
                    # trailing -1 block; keep the trim boundary lane-aligned)
                    c16 = min(_ceil(cnt, 16) * 16, nw)
                    idx[off + c16:off + nw] = -1
                # (mid-call pads stay 0: re-gather bucket row 0, harmless)
                sl = toff * P + off + np.arange(cnt)  # slot index within pair
                d = dstid[lo:hi].astype(np.int64)
                dstp[sl % P, tb + sl // P] = d.astype(BF16)
                ohd1[d, tb + sl // P, sl % P] = ml_dtypes.float8_e4m3(1.0)
                off += nw
            g1i[:, icol1:icol1 + n // 16] = wrap16(idx)
    return g1i, ohd1, dstp


def pack_xt(cfg: Cfg, X: np.ndarray) -> np.ndarray:
    """X [N, HID] f32 -> bf16 packed [KP, NT, KT, P]: (p, j, ki, n) = X[j*P+n, ki*KP+p]."""
    Xp = np.zeros((cfg.NPAD, cfg.HID), np.float32)
    Xp[: cfg.N] = X
    Xb = Xp.astype(BF16)
    v = Xb.reshape(cfg.NT, P, cfg.KT, cfg.KP)
    return np.ascontiguousarray(v.transpose(3, 0, 2, 1))


def pack_w(cfg: Cfg, W, a_src, a_tgt):
    """Returns wt [KP, KT, HID] bf16 and wa [KP, KT, 2*HEADS] bf16."""
    WT = W.T.astype(np.float32)                       # [HID(d), HID(o)]
    wa_s = (W.reshape(cfg.HEADS, cfg.F, cfg.HID)
            * np.asarray(a_src, np.float32).reshape(cfg.HEADS, cfg.F, 1)).sum(1)
    wa_t = (W.reshape(cfg.HEADS, cfg.F, cfg.HID)
            * np.asarray(a_tgt, np.float32).reshape(cfg.HEADS, cfg.F, 1)).sum(1)
    WA = np.concatenate([wa_s.T, wa_t.T], axis=1)     # [d, 2H]
    wt = np.ascontiguousarray(
        WT.astype(BF16).reshape(cfg.KT, cfg.KP, cfg.HID).transpose(1, 0, 2))
    wa = np.ascontiguousarray(
        WA.astype(BF16).reshape(cfg.KT, cfg.KP, 2 * cfg.HEADS).transpose(1, 0, 2))
    return wt, wa


def _bcast_last(ap, n):
    """Append a 0-stride broadcast dim of size n to an AP."""
    import concourse.bass as bass
    lst = [list(x) for x in ap.ap] + [[0, n]]
    return bass.AP(ap.tensor, ap.offset, lst)


def _mid_bcast(ap, n):
    """Insert a 0-stride dim of size n after the partition dim of a 2D AP."""
    import concourse.bass as bass
    lst = [list(ap.ap[0]), [0, n]] + [list(x) for x in ap.ap[1:]]
    return bass.AP(ap.tensor, ap.offset, lst)


def build_nc(cfg: Cfg, sched: Schedule, reps: int = 1, skip=()):
    """skip: subset of {"p1", "gather", "oh", "mm"} — timing-only variants."""
    import concourse.bacc as bacc
    import concourse.bass as bass
    import concourse.mybir as mybir
    from concourse.tile import TileContext

    dt = mybir.dt
    H, HID, KT, KP = cfg.HEADS, cfg.HID, cfg.KT, cfg.KP

    nc = bacc.Bacc("TRN2", target_bir_lowering=False)

    xt = nc.dram_tensor("xt", [KP, cfg.NT, KT, P], dt.bfloat16, kind="ExternalInput")
    wt = nc.dram_tensor("wt", [KP, KT, HID], dt.bfloat16, kind="ExternalInput")
    wa = nc.dram_tensor("wa", [KP, KT, 2 * H], dt.bfloat16, kind="ExternalInput")
    g1i = nc.dram_tensor("g1i", [P, sched.idxcols1], dt.int16, kind="ExternalInput")
    ohd1 = nc.dram_tensor("ohd1", [P, sched.TT, P], dt.float8e4,
                          kind="ExternalInput")
    dstp = nc.dram_tensor("dstp", [P, sched.TT], dt.bfloat16, kind="ExternalInput")
    avec = nc.dram_tensor("avec", [P, 1], dt.float32, kind="ExternalInput")
    out = nc.dram_tensor("out", [cfg.NW * P, HID], dt.float32, kind="ExternalOutput")

    with TileContext(nc) as tc:
        with tc.tile_pool(name="const", bufs=1) as cpool, \
             tc.tile_pool(name="dram", bufs=1, space="DRAM") as dpool:
            table = dpool.tile([cfg.NPAD, cfg.row_bf], dt.bfloat16)
            wt_sb = cpool.tile([KP, KT, HID], dt.bfloat16)
            nc.sync.dma_start(out=wt_sb[:], in_=wt[:, :, :])
            wa_sb = cpool.tile([KP, KT, 2 * H], dt.bfloat16)
            nc.sync.dma_start(out=wa_sb[:], in_=wa[:, :, :])
            a_sb = cpool.tile([P, 1], dt.float32)
            nc.sync.dma_start(out=a_sb[:], in_=avec[:, :])
            g1i_sb = cpool.tile([P, sched.idxcols1], dt.int16)
            nc.sync.dma_start(out=g1i_sb[:], in_=g1i[:, :])
            dstp_sb = cpool.tile([P, sched.TT], dt.bfloat16)
            nc.sync.dma_start(out=dstp_sb[:], in_=dstp[:, :])
            pid = nc.sync.partition_id()
            # iota ramp 0..127 in bf16, identical in every partition
            iota_i16 = cpool.tile([P, P], dt.int16)
            nc.gpsimd.iota(iota_i16[:], pattern=[[1, P]], base=0,
                           channel_multiplier=0)
            iota_mat = cpool.tile([P, P], dt.bfloat16)
            nc.vector.tensor_copy(out=iota_mat[:], in_=iota_i16[:])

            for rep in range(reps):
                _emit_rep(cfg, sched, nc, tc, bass, mybir, dt, rep,
                          table, wt_sb, wa_sb, a_sb, g1i_sb,
                          dstp_sb, iota_mat, pid, xt, ohd1, out, skip)
                if rep < reps - 1:
                    tc.strict_bb_all_engine_barrier()

    nc.compile()
    return nc


def _emit_rep(cfg, sched, nc, tc, bass, mybir, dt, rep,
              table, wt_sb, wa_sb, a_sb, g1i_sb,
              dstp_sb, iota_mat, pid, xt, ohd1, out, skip):
    H, HID, KT, KP = cfg.HEADS, cfg.HID, cfg.KT, cfg.KP

    # ---------------- phase 1: projection table ----------------
    if "p1" not in skip:
        with tc.tile_pool(name=f"p1_{rep}", bufs=3) as xpool, \
             tc.tile_pool(name=f"p1ps_{rep}", bufs=2, space="PSUM") as pspool, \
             tc.tile_pool(name=f"p1st_{rep}", bufs=3) as stpool:
            for j in range(cfg.NT):
                xtile = xpool.tile([KP, KT, P], dt.bfloat16, tag="x")
                nc.sync.dma_start(out=xtile[:], in_=xt[:, j, :, :])
                ps1 = pspool.tile([P, HID], dt.float32, tag="ps1")
                ps2 = pspool.tile([P, 2 * H], dt.float32, tag="ps2")
                for ki in range(KT):
                    nc.tensor.matmul(ps1[:], xtile[:, ki, :], wt_sb[:, ki, :],
                                     start=(ki == 0), stop=(ki == KT - 1))
                for ki in range(KT):
                    nc.tensor.matmul(ps2[:], xtile[:, ki, :], wa_sb[:, ki, :],
                                     start=(ki == 0), stop=(ki == KT - 1))
                stg = stpool.tile([P, cfg.row_used], dt.bfloat16, tag="stg")
                stg32 = stg.bitcast(dt.float32)
                nc.scalar.copy(out=stg[:, 0:HID], in_=ps1[:])
                nc.scalar.copy(out=stg32[:, cfg.s_src_f32:cfg.s_src_f32 + 2 * H],
                               in_=ps2[:])
                nc.sync.dma_start(out=table[j * P:(j + 1) * P, 0:cfg.row_used],
                                  in_=stg[:])

        tc.strict_bb_all_engine_barrier()

    # ------- phase 1.5: resident s_tgt (hi/lo bf16) -------
    with tc.tile_pool(name=f"sres_{rep}", bufs=1) as spool:
        table32 = table.bitcast(dt.float32)
        s_ap = table32[bass.DynSlice(pid * cfg.shard, cfg.NW * P),
                       cfg.s_tgt_f32:cfg.s_tgt_f32 + H]
        s_ap = s_ap.rearrange("(w p) h -> p w h", p=P)
        s_all = spool.tile([P, cfg.NW, H], dt.float32)
        nc.sync.dma_start(out=s_all[:], in_=s_ap)
        s_hilo = spool.tile([P, cfg.NW, 2, H], dt.bfloat16)
        s_hi32 = spool.tile([P, cfg.NW, H], dt.float32)
        nc.vector.tensor_copy(out=s_hilo[:, :, 0, :], in_=s_all[:])
        nc.vector.tensor_copy(out=s_hi32[:], in_=s_hilo[:, :, 0, :])
        nc.vector.tensor_tensor(out=s_hilo[:, :, 1, :], in0=s_all[:],
                                in1=s_hi32[:], op=mybir.AluOpType.subtract)

        # ---------------- phase 2: window pairs ----------------
        with tc.tile_pool(name=f"p2_{rep}", bufs=2) as pool, \
             tc.tile_pool(name=f"p2ps_{rep}", bufs=2, space="PSUM") as pps:
            for pr in range(cfg.NP2):
                T2 = sched.T2[pr]
                tb = sched.tile_base[pr]
                g1t = pool.tile([P, sched.T2max, cfg.row_bf], dt.bfloat16,
                                tag="g1t", bufs=3)
                if pr < 2:
                    nc.vector.memset(g1t[:], 0.0)  # finite stale for -1 pads
                if "gather" not in skip:
                    for (b, toff, groups, icol1) in sched.g1calls[pr]:
                        n = sum(x[1] for x in groups)
                        rows = min(cfg.NPAD, (b + 1) * cfg.bucket) - b * cfg.bucket
                        nc.gpsimd.dma_gather(
                            g1t[:, toff:toff + n // P, :],
                            table[b * cfg.bucket:b * cfg.bucket + rows, :],
                            g1i_sb[:, icol1:icol1 + n // 16], n, n, cfg.row_bf)
                else:
                    nc.vector.memset(g1t[:, 0:1, :] if pr >= 2 else g1t[:], 0.0)
                oht = pool.tile([P, sched.T2max, P], dt.float8e4, tag="oht",
                                bufs=4)
                if "oh" not in skip:
                    nc.sync.dma_start(out=oht[:, :T2, :],
                                      in_=ohd1[:, tb:tb + T2, :])
                else:
                    ob = oht.bitcast(dt.bfloat16)
                    nc.vector.memset(ob[:, 0:1, :] if pr >= 2 else ob[:], 0.0)

                # one-hot [slot(p), tile, dst] = (dstp[p, tile] == iota[dst])
                oh0 = pool.tile([P, sched.T2max, P], dt.bfloat16, tag="oh0")
                if "oh" not in skip:
                    in0 = _bcast_last(dstp_sb[:, tb:tb + T2], P)
                    in1 = _mid_bcast(iota_mat[:], T2)
                    nc.vector.tensor_tensor(out=oh0[:, :T2, :], in0=in0, in1=in1,
                                            op=mybir.AluOpType.is_equal)
                else:
                    nc.vector.memset(oh0[:, 0:1, :] if pr >= 2 else oh0[:], 0.0)

                stgt = pps.tile([P, sched.T2max, 2, H], dt.float32, tag="stgt")
                if "mm" not in skip:
                    for t in range(T2):
                        w = sched.tinfo[tb + t][0]
                        nc.tensor.matmul(stgt[:, t, :, :], oht[:, t, :],
                                         s_hilo[:, w, :, :], start=True,
                                         stop=True)
                else:
                    nc.vector.memset(
                        stgt[:, 0:1, :, :] if pr >= 2 else stgt[:], 0.0)
                g1t32 = g1t.bitcast(dt.float32)
                s_sum = pool.tile([P, sched.T2max, H], dt.float32, tag="s_sum")
                s_act = pool.tile([P, sched.T2max, H], dt.float32, tag="s_act")
                nc.vector.tensor_tensor(
                    out=s_sum[:, :T2, :],
                    in0=g1t32[:, :T2, cfg.s_src_f32:cfg.s_src_f32 + H],
                    in1=stgt[:, :T2, 0, :], op=mybir.AluOpType.add)
                nc.vector.tensor_tensor(
                    out=s_act[:, :T2, :], in0=s_sum[:, :T2, :],
                    in1=stgt[:, :T2, 1, :], op=mybir.AluOpType.add)
                nc.vector.scalar_tensor_tensor(
                    out=s_act[:, :T2, :], in0=s_act[:, :T2, :], scalar=cfg.leak,
                    in1=s_act[:, :T2, :], op0=mybir.AluOpType.mult,
                    op1=mybir.AluOpType.max)
                exp_t = pool.tile([P, sched.T2max, H], dt.bfloat16, tag="exp_t")
                nc.scalar.activation(out=exp_t[:, :T2, :], in_=s_act[:, :T2, :],
                                     func=mybir.ActivationFunctionType.Exp)

                w_t = pool.tile([P, sched.T2max, HID], dt.bfloat16, tag="w_t")
                proj4 = g1t[:, :T2, 0:HID].rearrange("p t (h f) -> p t h f", h=H)
                exp4 = _bcast_last(exp_t[:, :T2, :], cfg.F)
                out4 = w_t[:, :T2, :].rearrange("p t (h f) -> p t h f", h=H)
                nc.vector.tensor_tensor(out=out4, in0=proj4, in1=exp4,
                                        op=mybir.AluOpType.mult)

                for wi in range(2):
                    w = 2 * pr + wi
                    tiles_w = sched.wtiles[w]
                    agg = pps.tile([P, HID], dt.float32, tag="agg", bufs=3)
                    den = pps.tile([P, H], dt.float32, tag="den", bufs=2)
                    if "mm" not in skip and tiles_w:
                        for i, lt in enumerate(tiles_w):
                            nc.tensor.matmul(agg[:], oh0[:, lt, :], w_t[:, lt, :],
                                             start=(i == 0),
                                             stop=(i == len(tiles_w) - 1))
                            nc.tensor.matmul(den[:], oh0[:, lt, :],
                                             exp_t[:, lt, :],
                                             start=(i == 0),
                                             stop=(i == len(tiles_w) - 1))
                    else:
                        nc.vector.memset(agg[:, 0:4] if pr >= 1 else agg[:], 0.0)
                        nc.vector.memset(den[:, 0:4] if pr >= 1 else den[:], 0.0)

                    den_sb = pool.tile([P, H, 1], dt.float32, tag="den_sb")
                    recip = pool.tile([P, H, 1], dt.float32, tag="recip")
                    nc.vector.tensor_scalar_add(out=den_sb[:, :, 0], in0=den[:],
                                                scalar1=1e-16)
                    nc.vector.reciprocal(out=recip[:], in_=den_sb[:])
                    z = pool.tile([P, HID], dt.float32, tag="z")
                    agg4 = agg[:].rearrange("p (h f) -> p h f", h=H)
                    z4 = z[:].rearrange("p (h f) -> p h f", h=H)
                    nc.vector.tensor_tensor(out=z4, in0=agg4,
                                            in1=_bcast_last(recip[:, :, 0], cfg.F),
                                            op=mybir.AluOpType.mult)
                    res = pool.tile([P, HID], dt.float32, tag="res")
                    nc.vector.scalar_tensor_tensor(
                        out=res[:], in0=z[:], scalar=a_sb[:, 0:1], in1=z[:],
                        op0=mybir.AluOpType.mult, op1=mybir.AluOpType.max)
                    nc.sync.dma_start(out=out[w * P:(w + 1) * P, :], in_=res[:])


def prepare(cfg: Cfg, inputs):
    """Host-side prep shared by HW and sim paths.

    Returns (sched, in_maps, assemble) where assemble(core_outs) -> full out.
    """
    X = np.asarray(inputs["in_nodes_features"], np.float32)
    ei = np.asarray(inputs["edge_index"], np.int64)
    W = np.asarray(inputs["W"], np.float32)
    b_lin = np.asarray(inputs["b_lin"], np.float32)
    a_src = np.asarray(inputs["a_src"], np.float32)
    a_tgt = np.asarray(inputs["a_tgt"], np.float32)
    bias = np.asarray(inputs["bias"], np.float32)
    prelu_a = float(np.asarray(inputs["prelu_a"], np.float32))

    assert np.all(b_lin == 0) and np.all(bias == 0), "nonzero bias unsupported"
    assert 0.0 <= prelu_a <= 1.0, "prelu_a outside [0,1] unsupported"

    src, trg = ei[0], ei[1]
    core_of = trg // cfg.shard
    win_of = (trg % cfg.shard) // P
    buck_of = src // cfg.bucket
    counts = np.zeros((cfg.ncores, cfg.NW, cfg.NB), np.int64)
    for k in range(cfg.ncores):
        m = core_of == k
        counts[k] = np.bincount(
            win_of[m] * cfg.NB + buck_of[m],
            minlength=cfg.NW * cfg.NB).reshape(cfg.NW, cfg.NB)
    sched = build_schedule(cfg, counts)

    xt = pack_xt(cfg, X)
    wtp, wap = pack_w(cfg, W, a_src, a_tgt)
    av = np.full((P, 1), prelu_a, np.float32)

    in_maps = []
    for k in range(cfg.ncores):
        g1i_k, ohd1_k, dstp_k = prep_core(cfg, sched, src, trg, k)
        in_maps.append({
            "xt": xt, "wt": wtp, "wa": wap,
            "g1i": g1i_k, "ohd1": ohd1_k, "dstp": dstp_k, "avec": av,
        })

    def assemble(core_outs):
        return np.concatenate(
            [np.asarray(o["out"][: cfg.shard], np.float32) for o in core_outs], axis=0)

    return sched, in_maps, assemble


_BUILT = {}


def _get_built(cfg: Cfg, sched: Schedule):
    key = (cfg.N, cfg.E, cfg.HID, cfg.HEADS, cfg.ncores, cfg.bucket,
           tuple(sched.TW), sched.idxcols1, sched.idxcols2)
    if key not in _BUILT:
        _BUILT[key] = build_nc(cfg, sched)
    return _BUILT[key]


def kernel(**inputs):
    from concourse.bass_utils import run_bass_kernel_spmd

    cfg = Cfg()
    sched, in_maps, assemble = prepare(cfg, inputs)
    nc = _get_built(cfg, sched)
    res = run_bass_kernel_spmd(nc, in_maps, core_ids=list(range(cfg.ncores)))
    return assemble(res.results)
